# revision 29
# baseline (speedup 1.0000x reference)
"""Committee-vote histogram kernel for TRN2 (8 NeuronCores, data-parallel).

votes[b, c] = sum_m 1[argmax_c' (x[b] @ W[m, :, c'] + b[m, c']) == c]

Strategy per core (batch shard of 8192 rows):
  - x is decomposed host-side into an exact fp16 pair (x = xh + xl with
    residual ~2^-22|x|); likewise W. Logits are computed as
    xh@Wh + xh@Wl + xl@Wh (+bias), whose decomposition error (~2e-7) is at
    fp32 rounding level - validated exact-match against the fp32 reference.
  - The host packs each core's x halves into ONE array [128, 4*8192] fp16
    whose rows mirror the SBUF chunk tiles exactly (per chunk: [k, h, b]
    blocks), so every chunk DMA is 128 contiguous multi-KB descriptors.
    k=0 halves stream on the sync HWDGE queue, k=1 on the scalar queue, so
    the PE's k-phased pipeline starts after the first 0.25 MB lands.
  - The first transfer on each queue ramps slowly, so chunk0 (which
    covers all of super-batch 0) is split into (k, h) quarters - k0
    halved again, consts riding between the halves - so SB0's first
    tiles gate on ~340KB of joint queue drain. While it is in flight
    the PE runs warm-up matmuls (p-state clock ramp); bias seeds (K=2
    ones @ (bh|bl) matmuls, exact f32 sum) hide inside SB0's known
    data-wait stalls, and banks 7-15 seed in the main loop on reuse.
    Middle chunks are 2048 rows so each chunk-boundary DMA-completion
    lag (~2.5us) hides behind >=3us of per-half matmul work.
  - Logits accumulate per 4-tile GROUP into single-bank PSUM tiles (7 in
    flight) so PSUM turnover is fine-grained and the PE never stalls on
    bank reuse. The steady state is LDWEIGHTS-bound (all of x flows
    through the PE weight port at ~2 loads per tile and k-half).
  - The two xh passes are FUSED into one matmul per (tile, k): rhs is the
    concatenated (wh|wl) 160 columns and the out AP carries a stride-0
    dim that folds columns 80..159 back onto 0..79, so both products
    accumulate into the same PSUM cells (start=False => every column
    write accumulates).
  - Votes: per group, DVE does reduce_max over c then an is_ge mask into
    fp16 (the second op frees the PSUM bank); the member-sum add-tree
    (exact in fp16 for counts <= 8) runs on the otherwise-idle GpSimd
    engine per super-batch, straight into the staging tile. Each super-
    batch's votes are stored with their own small DMA so only the last
    ~10KB store sits on the tail; the final super-batch runs per-group
    DVE-only chains to shorten the tail.
  - The host de-interleaves [p, sb, g, t, c] -> [b, c] and casts to f32.
"""

import os
import sys

import numpy as np

if os.path.isdir("/opt/trn_rl_repo") and "/opt/trn_rl_repo" not in sys.path:
    sys.path.insert(0, "/opt/trn_rl_repo")

import concourse.bass as bass
import concourse.tile as tile
from concourse import bacc, mybir

F32 = mybir.dt.float32
F16 = mybir.dt.float16

B_FULL = 65536
D = 256
C = 10
M = 8
N_CORES = 8
B_SHARD = B_FULL // N_CORES  # 8192
P = 128

MC = M * C  # 80 logit columns per sample
CHUNKS = (512, 512, 2048, 2048, 2048, 512, 512)  # rows per input-DMA chunk
WARMUP_MMS = 6


def build_nc(b_shard: int = B_SHARD) -> bass.Bass:
    assert sum(CHUNKS) == b_shard
    n_tiles = b_shard // P  # 64
    n_sb = n_tiles // 8  # 8 super-batches of 8 tiles (2 groups x 4 tiles)
    n_gr = n_tiles // 4  # 16 four-tile groups (one PSUM bank each)

    nc = bacc.Bacc("TRN2", target_bir_lowering=False)
    # packed x halves: per chunk c with L rows at column base, cols
    # [4*base + (k*2 + h)*L + 0..L) hold half (k, h) in [d-in-k, b] layout
    xin = nc.dram_tensor("xin", [P, 4 * b_shard], F16, kind="ExternalInput")
    whl = nc.dram_tensor("whl", [P, 4 * MC], F16, kind="ExternalInput")
    bc2 = nc.dram_tensor("bc2", [2, 4 * MC], F16, kind="ExternalInput")
    # votes in SBUF staging layout [p, sb*80 + g*40 + t*10 + c]
    y = nc.dram_tensor("y", [P, n_tiles * C], F16, kind="ExternalOutput")

    with tile.TileContext(nc) as tc:
        with (
            tc.tile_pool(name="consts", bufs=1) as consts,
            tc.tile_pool(name="xt", bufs=len(CHUNKS)) as xt_pool,
            tc.tile_pool(name="warm", bufs=1, space="PSUM") as warm_pool,
            tc.tile_pool(name="lg", bufs=7, space="PSUM") as lg_pool,
            tc.tile_pool(name="mx", bufs=4) as mx_pool,
            tc.tile_pool(name="eq", bufs=3) as eq_pool,
            tc.tile_pool(name="tsum", bufs=2) as tsum_pool,
            tc.tile_pool(name="stg", bufs=1) as stg_pool,
        ):
            # --- DMA issue. The first transfer on each queue ramps slowly
            # (~55-100 GB/s), so chunk0 (which covers all of SB0) is split
            # into (k, h) quarters with xh on sync and xl on scalar - both
            # queues deliver SB0's first data in parallel. bc2 (tiny) heads
            # the sync queue, whl the scalar queue.
            bc2_sb = consts.tile([2, 4 * MC], F16)
            whl_sb = consts.tile([P, 2, 2, MC], F16)
            xts = [
                xt_pool.tile([P, 2, 2, L], F16, name="xt") for L in CHUNKS
            ]
            L0 = CHUNKS[0]
            # chunks 0 and 1 are one 4-tile group each; both k halves are
            # h-split across the queues so group 0 is fully computable
            # after ~0.5MB of joint drain and group 1 after ~1MB. The
            # consts ride between the first pieces.
            nc.sync.dma_start(xts[0][:, 0, 0], xin[:, 0:L0])
            nc.scalar.dma_start(xts[0][:, 0, 1], xin[:, L0 : 2 * L0])
            nc.sync.dma_start(bc2_sb, bc2[:])
            nc.scalar.dma_start(
                whl_sb, whl.rearrange("p (k h c) -> p k h c", k=2, h=2)
            )
            nc.sync.dma_start(xts[0][:, 1, 0], xin[:, 2 * L0 : 3 * L0])
            nc.scalar.dma_start(xts[0][:, 1, 1], xin[:, 3 * L0 : 4 * L0])
            o1 = 4 * L0
            L1 = CHUNKS[1]
            for k in range(2):
                nc.sync.dma_start(
                    xts[1][:, k, 0],
                    xin[:, o1 + 2 * k * L1 : o1 + (2 * k + 1) * L1],
                )
                nc.scalar.dma_start(
                    xts[1][:, k, 1],
                    xin[:, o1 + (2 * k + 1) * L1 : o1 + (2 * k + 2) * L1],
                )
            base = CHUNKS[0] + CHUNKS[1]
            for ci, L in list(enumerate(CHUNKS))[2:]:
                nc.sync.dma_start(
                    xts[ci][:, 0],
                    xin[:, 4 * base : 4 * base + 2 * L].rearrange(
                        "p (h l) -> p h l", h=2
                    ),
                )
                base += L
            base = CHUNKS[0] + CHUNKS[1]
            for ci, L in list(enumerate(CHUNKS))[2:]:
                nc.scalar.dma_start(
                    xts[ci][:, 1],
                    xin[:, 4 * base + 2 * L : 4 * base + 4 * L].rearrange(
                        "p (h l) -> p h l", h=2
                    ),
                )
                base += L

            lgs = [
                lg_pool.tile([P, 512], F32, name="lg") for _ in range(n_gr)
            ]
            ones_g = consts.tile([P, 512], F16)
            nc.gpsimd.memset(ones_g, 1.0)

            def seed_bias(gr):
                # seed the bank with the bias: every row of ones.T @ (bh4|
                # bl4) is bh4+bl4, summed exactly in f32 PSUM
                nc.tensor.matmul(
                    lgs[gr][:, : 4 * MC], lhsT=ones_g[:2, :P], rhs=bc2_sb,
                    start=True, stop=False,
                )

            # --- PE warm-up while the first chunk is in flight; bias
            # seeds all run in the main loop (2 per super-batch) so the
            # cold-clock prologue stays short
            warm = warm_pool.tile([P, 512], F32)
            for _ in range(WARMUP_MMS):
                nc.tensor.matmul(
                    warm, lhsT=ones_g[:, :P], rhs=ones_g, start=True, stop=True
                )

            stg = stg_pool.tile([P, n_tiles * C], F16)

            # global tile T -> (chunk index, within-chunk column)
            tile_map = []
            for ci, L in enumerate(CHUNKS):
                for t in range(L // P):
                    tile_map.append((ci, t * P))

            def vote_mask(gr):
                """reduce_max + is_ge for one group; frees its PSUM bank.
                Returns the eq mask slice [P, 4, M, C] (fp16)."""
                lg = lgs[gr]
                lgv = lg[:, : 4 * MC].rearrange("p (t m c) -> p t m c", m=M, c=C)
                mx = mx_pool.tile([P, 4, M], F32, name="mx")
                nc.vector.reduce_max(mx, lgv, axis=mybir.AxisListType.X)
                eq = eqs[gr // 2][:, gr % 2]
                nc.vector.tensor_tensor(
                    out=eq,
                    in0=lgv,
                    in1=mx[:, :, :, None].broadcast_to([P, 4, M, C]),
                    op=mybir.AluOpType.is_ge,
                )
                return eq

            # --- main pipeline: super-batches of 8 tiles (2 groups) ---
            eqs = []
            for SB in range(n_sb):
                lgA, lgB = lgs[2 * SB], lgs[2 * SB + 1]
                eqs.append(eq_pool.tile([P, 2, 4, M, C], F16, name="eq"))
                if SB == 0:
                    seed_bias(0)
                    seed_bias(1)
                else:
                    for gr in (2 * SB, 2 * SB + 1):
                        if gr >= 7:
                            seed_bias(gr)
                # SB0 runs group-serially (each group's data is its own
                # early chunk); later SBs are k-phased so phase 0 only
                # needs the k=0 x halves
                sb0_phases = [(k, j) for g in range(2) for k in range(2)
                              for j in range(g * 4, g * 4 + 4)]
                phases = (
                    sb0_phases if SB == 0 else
                    [(k, j) for k in range(2) for j in range(8)]
                )
                for pi, (k, j) in enumerate(phases):
                        lg = lgA if j < 4 else lgB
                        o = (j % 4) * MC
                        ci, col = tile_map[SB * 8 + j]
                        xt = xts[ci]
                        xh_c = xt[:, k, 0, col : col + P]
                        xl_c = xt[:, k, 1, col : col + P]
                        out = lg[:, o : o + MC]
                        last = k == 1 and (j % 4) == 3
                        # xh@wh + xh@wl in ONE matmul: the out AP's
                        # stride-0 h dim folds columns 80..159 onto
                        # 0..79, accumulating both products (start=False
                        # means every column-write accumulates)
                        nc.tensor.matmul(
                            out[:, None, :].broadcast_to([P, 2, MC]),
                            lhsT=xh_c, rhs=whl_sb[:, k],
                            start=False, stop=False,
                        )
                        nc.tensor.matmul(
                            out, lhsT=xl_c, rhs=whl_sb[:, k, 0, :],
                            start=False, stop=last,
                        )
                        if SB == 0 and pi == 3:
                            # these seeds hide in the wait for group 0's
                            # k1 halves
                            seed_bias(2)
                            seed_bias(3)
                        if SB == 0 and pi == 7:
                            # and these in the wait for group 1's data
                            for gr in (4, 5, 6):
                                seed_bias(gr)

                if SB < n_sb - 1:
                    # per-group mask chains (each frees its PSUM bank);
                    # member-sum add-tree on the idle GpSimd engine
                    eqv = eqs[SB][:]
                    vote_mask(2 * SB)
                    vote_mask(2 * SB + 1)
                    t4 = tsum_pool.tile([P, 2, 4, 4, C], F16, name="t4")
                    nc.gpsimd.tensor_tensor(
                        out=t4,
                        in0=eqv[:, :, :, 0:4, :], in1=eqv[:, :, :, 4:8, :],
                        op=mybir.AluOpType.add,
                    )
                    t2 = tsum_pool.tile([P, 2, 4, 2, C], F16, name="t2")
                    nc.gpsimd.tensor_tensor(
                        out=t2,
                        in0=t4[:, :, :, 0:2, :], in1=t4[:, :, :, 2:4, :],
                        op=mybir.AluOpType.add,
                    )
                    nc.gpsimd.tensor_tensor(
                        out=stg[:, SB * 8 * C : (SB + 1) * 8 * C].rearrange(
                            "p (g t c) -> p g t c", g=2, c=C
                        ),
                        in0=t2[:, :, :, 0, :], in1=t2[:, :, :, 1, :],
                        op=mybir.AluOpType.add,
                    )
                    if SB % 2 == 1:
                        # store per SB-pair, alternating queues so no store
                        # waits behind another's issue on one engine
                        eng = nc.scalar if SB % 4 == 1 else nc.sync
                        eng.dma_start(
                            y[:, (SB - 1) * 8 * C : (SB + 1) * 8 * C],
                            stg[:, (SB - 1) * 8 * C : (SB + 1) * 8 * C],
                        )
                else:
                    nc.scalar.dma_start(
                        y[:, (SB - 1) * 8 * C : SB * 8 * C],
                        stg[:, (SB - 1) * 8 * C : SB * 8 * C],
                    )
                    # final super-batch: all-DVE per-group chains + split
                    # stores; the very last group masks at 2-tile grain so
                    # the final chain overlaps the last matmuls
                    for g in range(2):
                        if g == 0:
                            eq = vote_mask(2 * SB)
                        else:
                            gr = 2 * SB + 1
                            lg = lgs[gr]
                            eq = eqs[SB][:, 1]
                            for hf in range(2):
                                lgv = lg[
                                    :, hf * 2 * MC : (hf + 1) * 2 * MC
                                ].rearrange(
                                    "p (t m c) -> p t m c", m=M, c=C
                                )
                                mxf = mx_pool.tile([P, 2, M], F32, name="mxf")
                                nc.vector.reduce_max(
                                    mxf, lgv, axis=mybir.AxisListType.X
                                )
                                nc.vector.tensor_tensor(
                                    out=eq[:, hf * 2 : hf * 2 + 2],
                                    in0=lgv,
                                    in1=mxf[:, :, :, None].broadcast_to(
                                        [P, 2, M, C]
                                    ),
                                    op=mybir.AluOpType.is_ge,
                                )
                        t4 = tsum_pool.tile([P, 4, 4, C], F16, name="t4f")
                        nc.vector.tensor_tensor(
                            out=t4,
                            in0=eq[:, :, 0:4, :], in1=eq[:, :, 4:8, :],
                            op=mybir.AluOpType.add,
                        )
                        t2 = tsum_pool.tile([P, 4, 2, C], F16, name="t2f")
                        nc.vector.tensor_tensor(
                            out=t2,
                            in0=t4[:, :, 0:2, :], in1=t4[:, :, 2:4, :],
                            op=mybir.AluOpType.add,
                        )
                        o = SB * 8 * C + g * 4 * C
                        nc.vector.tensor_tensor(
                            out=stg[:, o : o + 4 * C].rearrange(
                                "p (t c) -> p t c", c=C
                            ),
                            in0=t2[:, :, 0, :], in1=t2[:, :, 1, :],
                            op=mybir.AluOpType.add,
                        )
                        # g0's store on scalar, g1's (the last) on sync -
                        # the two final receipts overlap
                        eng = nc.scalar if g == 0 else nc.sync
                        eng.dma_start(
                            y[:, o : o + 4 * C], stg[:, o : o + 4 * C]
                        )
    nc.compile()
    return nc


_NC_CACHE: dict[int, bass.Bass] = {}


def _get_nc(b_shard: int) -> bass.Bass:
    if b_shard not in _NC_CACHE:
        _NC_CACHE[b_shard] = build_nc(b_shard)
    return _NC_CACHE[b_shard]


def make_in_maps(x: np.ndarray, W: np.ndarray, b: np.ndarray):
    """Host-side prep: exact fp16 pair decomposition + per-core packing."""
    xf = np.asarray(x, dtype=np.float32)
    xh = xf.astype(np.float16)
    xl = (xf - xh.astype(np.float32)).astype(np.float16)
    # m-major columns: col index = 10*m + c; wh|wl concatenated per row
    wf = (
        np.asarray(W, dtype=np.float32).transpose(1, 0, 2).reshape(D, MC)
    )
    whf = wf.astype(np.float16)
    wlf = (wf - whf.astype(np.float32)).astype(np.float16)
    # packed [p, k, h, c]: row p holds (k0h0, k0h1, k1h0, k1h1) blocks
    whlf = np.empty((P, 4 * MC), dtype=np.float16)
    for k in range(2):
        for h, half in enumerate((whf, wlf)):
            whlf[:, (k * 2 + h) * MC : (k * 2 + h + 1) * MC] = half[
                k * P : (k + 1) * P
            ]
    bv = np.asarray(b, dtype=np.float32).reshape(MC)  # bv[10m+c] = b[m,c]
    bh = bv.astype(np.float16)
    bl = (bv - bh.astype(np.float32)).astype(np.float16)
    bc2 = np.ascontiguousarray(
        np.stack([np.tile(bh, 4), np.tile(bl, 4)], axis=0)
    ).astype(np.float16)

    xins = np.empty((N_CORES, P, 4 * B_SHARD), dtype=np.float16)
    halves = (xh, xl)
    for i in range(N_CORES):
        r0 = i * B_SHARD
        base = 0
        for L in CHUNKS:
            for k in range(2):
                for h in range(2):
                    c0 = 4 * base + (k * 2 + h) * L
                    xins[i, :, c0 : c0 + L] = halves[h][
                        r0 + base : r0 + base + L, k * P : (k + 1) * P
                    ].T
            base += L
    return [
        {"xin": xins[i], "whl": whlf, "bc2": bc2} for i in range(N_CORES)
    ]


def _postprocess(y_raw: np.ndarray) -> np.ndarray:
    # [p, (sb g t) * 10] fp16 -> [tile*128, 10] f32 (small ints: exact)
    n_tiles = y_raw.shape[1] // C
    return (
        y_raw.reshape(P, n_tiles, C)
        .transpose(1, 0, 2)
        .reshape(n_tiles * P, C)
        .astype(np.float32)
    )


def kernel(x: np.ndarray, W: np.ndarray, b: np.ndarray, **_) -> np.ndarray:
    from concourse.bass_utils import run_bass_kernel_spmd

    assert x.shape == (B_FULL, D), x.shape
    in_maps = make_in_maps(x, W, b)
    nc = _get_nc(B_SHARD)
    res = run_bass_kernel_spmd(nc, in_maps, core_ids=list(range(N_CORES)))
    return np.concatenate(
        [_postprocess(res.results[i]["y"]) for i in range(N_CORES)], axis=0
    )


# revision 30
# speedup vs baseline: 1.0158x; 1.0158x over previous
"""Committee-vote histogram kernel for TRN2 (8 NeuronCores, data-parallel).

votes[b, c] = sum_m 1[argmax_c' (x[b] @ W[m, :, c'] + b[m, c']) == c]

Strategy per core (batch shard of 8192 rows):
  - x is decomposed host-side into an exact fp16 pair (x = xh + xl with
    residual ~2^-22|x|); likewise W. Logits are computed as
    xh@Wh + xh@Wl + xl@Wh (+bias), whose decomposition error (~2e-7) is at
    fp32 rounding level - validated exact-match against the fp32 reference.
  - The host packs each core's x halves into ONE array [128, 4*8192] fp16
    whose rows mirror the SBUF chunk tiles exactly (per chunk: [k, h, b]
    blocks), so every chunk DMA is 128 contiguous multi-KB descriptors.
    k=0 halves stream on the sync HWDGE queue, k=1 on the scalar queue, so
    the PE's k-phased pipeline starts after the first 0.25 MB lands.
  - The first transfer on each queue ramps slowly, so the first two
    chunks are one 4-tile group each with BOTH k halves h-split across
    the queues (consts riding between the pieces): group 0 is fully
    computable after ~0.5MB of joint queue drain and group 1 after
    ~1MB, and super-batch 0 is emitted group-serially to match. While
    the data is in flight the PE runs warm-up matmuls (p-state clock
    ramp); bias seeds (K=2 ones @ (bh|bl) matmuls, exact f32 sum) hide
    inside SB0's known data-wait stalls, and banks 7-15 seed in the
    main loop on reuse. Middle chunks are 2048 rows so each chunk-
    boundary DMA-completion lag (~2.5us) hides behind >=3us of
    per-half matmul work; the last two chunks are small so the final
    super-batch's data lands early.
  - Logits accumulate per 4-tile GROUP into single-bank PSUM tiles (7 in
    flight) so PSUM turnover is fine-grained and the PE never stalls on
    bank reuse. The steady state is LDWEIGHTS-bound (all of x flows
    through the PE weight port at ~2 loads per tile and k-half).
  - The two xh passes are FUSED into one matmul per (tile, k): rhs is the
    concatenated (wh|wl) 160 columns and the out AP carries a stride-0
    dim that folds columns 80..159 back onto 0..79, so both products
    accumulate into the same PSUM cells (start=False => every column
    write accumulates).
  - Votes: per group, DVE does reduce_max over c then an is_ge mask into
    fp16 (the second op frees the PSUM bank); the member-sum add-tree
    (exact in fp16 for counts <= 8) runs on the otherwise-idle GpSimd
    engine per super-batch, straight into the staging tile. Each super-
    batch's votes are stored with their own small DMA so only the last
    ~10KB store sits on the tail; the final super-batch runs per-group
    DVE-only chains to shorten the tail.
  - The host de-interleaves [p, sb, g, t, c] -> [b, c] and casts to f32.
"""

import os
import sys

import numpy as np

if os.path.isdir("/opt/trn_rl_repo") and "/opt/trn_rl_repo" not in sys.path:
    sys.path.insert(0, "/opt/trn_rl_repo")

import concourse.bass as bass
import concourse.tile as tile
from concourse import bacc, mybir

F32 = mybir.dt.float32
F16 = mybir.dt.float16

B_FULL = 65536
D = 256
C = 10
M = 8
N_CORES = 8
B_SHARD = B_FULL // N_CORES  # 8192
P = 128

MC = M * C  # 80 logit columns per sample
CHUNKS = (512, 512, 2048, 2048, 2048, 512, 512)  # rows per input-DMA chunk
WARMUP_MMS = 6


def build_nc(b_shard: int = B_SHARD) -> bass.Bass:
    assert sum(CHUNKS) == b_shard
    n_tiles = b_shard // P  # 64
    n_sb = n_tiles // 8  # 8 super-batches of 8 tiles (2 groups x 4 tiles)
    n_gr = n_tiles // 4  # 16 four-tile groups (one PSUM bank each)

    nc = bacc.Bacc("TRN2", target_bir_lowering=False)
    # packed x halves: per chunk c with L rows at column base, cols
    # [4*base + (k*2 + h)*L + 0..L) hold half (k, h) in [d-in-k, b] layout
    xin = nc.dram_tensor("xin", [P, 4 * b_shard], F16, kind="ExternalInput")
    whl = nc.dram_tensor("whl", [P, 4 * MC], F16, kind="ExternalInput")
    bc2 = nc.dram_tensor("bc2", [2, 4 * MC], F16, kind="ExternalInput")
    # votes in SBUF staging layout [p, sb*80 + g*40 + t*10 + c]
    y = nc.dram_tensor("y", [P, n_tiles * C], F16, kind="ExternalOutput")

    with tile.TileContext(nc) as tc:
        with (
            tc.tile_pool(name="consts", bufs=1) as consts,
            tc.tile_pool(name="xt", bufs=len(CHUNKS)) as xt_pool,
            tc.tile_pool(name="warm", bufs=1, space="PSUM") as warm_pool,
            tc.tile_pool(name="lg", bufs=7, space="PSUM") as lg_pool,
            tc.tile_pool(name="mx", bufs=4) as mx_pool,
            tc.tile_pool(name="eq", bufs=3) as eq_pool,
            tc.tile_pool(name="tsum", bufs=2) as tsum_pool,
            tc.tile_pool(name="stg", bufs=1) as stg_pool,
        ):
            # --- DMA issue. The first transfer on each queue ramps slowly
            # (~55-100 GB/s), so chunk0 (which covers all of SB0) is split
            # into (k, h) quarters with xh on sync and xl on scalar - both
            # queues deliver SB0's first data in parallel. bc2 (tiny) heads
            # the sync queue, whl the scalar queue.
            bc2_sb = consts.tile([2, 4 * MC], F16)
            whl_sb = consts.tile([P, 2, 2, MC], F16)
            xts = [
                xt_pool.tile([P, 2, 2, L], F16, name="xt") for L in CHUNKS
            ]
            L0 = CHUNKS[0]
            # chunks 0 and 1 are one 4-tile group each; both k halves are
            # h-split across the queues so group 0 is fully computable
            # after ~0.5MB of joint drain and group 1 after ~1MB. The
            # consts ride between the first pieces.
            nc.sync.dma_start(xts[0][:, 0, 0], xin[:, 0:L0])
            nc.scalar.dma_start(xts[0][:, 0, 1], xin[:, L0 : 2 * L0])
            nc.sync.dma_start(bc2_sb, bc2[:])
            nc.scalar.dma_start(
                whl_sb, whl.rearrange("p (k h c) -> p k h c", k=2, h=2)
            )
            nc.sync.dma_start(xts[0][:, 1, 0], xin[:, 2 * L0 : 3 * L0])
            nc.scalar.dma_start(xts[0][:, 1, 1], xin[:, 3 * L0 : 4 * L0])
            o1 = 4 * L0
            L1 = CHUNKS[1]
            for k in range(2):
                nc.sync.dma_start(
                    xts[1][:, k, 0],
                    xin[:, o1 + 2 * k * L1 : o1 + (2 * k + 1) * L1],
                )
                nc.scalar.dma_start(
                    xts[1][:, k, 1],
                    xin[:, o1 + (2 * k + 1) * L1 : o1 + (2 * k + 2) * L1],
                )
            base = CHUNKS[0] + CHUNKS[1]
            for ci, L in list(enumerate(CHUNKS))[2:]:
                nc.sync.dma_start(
                    xts[ci][:, 0],
                    xin[:, 4 * base : 4 * base + 2 * L].rearrange(
                        "p (h l) -> p h l", h=2
                    ),
                )
                base += L
            base = CHUNKS[0] + CHUNKS[1]
            for ci, L in list(enumerate(CHUNKS))[2:]:
                nc.scalar.dma_start(
                    xts[ci][:, 1],
                    xin[:, 4 * base + 2 * L : 4 * base + 4 * L].rearrange(
                        "p (h l) -> p h l", h=2
                    ),
                )
                base += L

            lgs = [
                lg_pool.tile([P, 512], F32, name="lg") for _ in range(n_gr)
            ]
            ones_g = consts.tile([P, 512], F16)
            nc.gpsimd.memset(ones_g, 1.0)

            def seed_bias(gr):
                # seed the bank with the bias: every row of ones.T @ (bh4|
                # bl4) is bh4+bl4, summed exactly in f32 PSUM
                nc.tensor.matmul(
                    lgs[gr][:, : 4 * MC], lhsT=ones_g[:2, :P], rhs=bc2_sb,
                    start=True, stop=False,
                )

            # --- PE warm-up while the first chunk is in flight; bias
            # seeds all run in the main loop (2 per super-batch) so the
            # cold-clock prologue stays short
            warm = warm_pool.tile([P, 512], F32)
            for _ in range(WARMUP_MMS):
                nc.tensor.matmul(
                    warm, lhsT=ones_g[:, :P], rhs=ones_g, start=True, stop=True
                )

            stg = stg_pool.tile([P, n_tiles * C], F16)

            # global tile T -> (chunk index, within-chunk column)
            tile_map = []
            for ci, L in enumerate(CHUNKS):
                for t in range(L // P):
                    tile_map.append((ci, t * P))

            def vote_mask(gr):
                """reduce_max + is_ge for one group; frees its PSUM bank.
                Returns the eq mask slice [P, 4, M, C] (fp16)."""
                lg = lgs[gr]
                lgv = lg[:, : 4 * MC].rearrange("p (t m c) -> p t m c", m=M, c=C)
                mx = mx_pool.tile([P, 4, M], F32, name="mx")
                nc.vector.reduce_max(mx, lgv, axis=mybir.AxisListType.X)
                eq = eqs[gr // 2][:, gr % 2]
                nc.vector.tensor_tensor(
                    out=eq,
                    in0=lgv,
                    in1=mx[:, :, :, None].broadcast_to([P, 4, M, C]),
                    op=mybir.AluOpType.is_ge,
                )
                return eq

            # --- main pipeline: super-batches of 8 tiles (2 groups) ---
            eqs = []
            for SB in range(n_sb):
                lgA, lgB = lgs[2 * SB], lgs[2 * SB + 1]
                eqs.append(eq_pool.tile([P, 2, 4, M, C], F16, name="eq"))
                if SB == 0:
                    seed_bias(0)
                    seed_bias(1)
                else:
                    for gr in (2 * SB, 2 * SB + 1):
                        if gr >= 7:
                            seed_bias(gr)
                # SB0 runs group-serially (each group's data is its own
                # early chunk); later SBs are k-phased so phase 0 only
                # needs the k=0 x halves
                sb0_phases = [(k, j) for g in range(2) for k in range(2)
                              for j in range(g * 4, g * 4 + 4)]
                phases = (
                    sb0_phases if SB == 0 else
                    [(k, j) for k in range(2) for j in range(8)]
                )
                for pi, (k, j) in enumerate(phases):
                        lg = lgA if j < 4 else lgB
                        o = (j % 4) * MC
                        ci, col = tile_map[SB * 8 + j]
                        xt = xts[ci]
                        xh_c = xt[:, k, 0, col : col + P]
                        xl_c = xt[:, k, 1, col : col + P]
                        out = lg[:, o : o + MC]
                        last = k == 1 and (j % 4) == 3
                        # xh@wh + xh@wl in ONE matmul: the out AP's
                        # stride-0 h dim folds columns 80..159 onto
                        # 0..79, accumulating both products (start=False
                        # means every column-write accumulates)
                        nc.tensor.matmul(
                            out[:, None, :].broadcast_to([P, 2, MC]),
                            lhsT=xh_c, rhs=whl_sb[:, k],
                            start=False, stop=False,
                        )
                        nc.tensor.matmul(
                            out, lhsT=xl_c, rhs=whl_sb[:, k, 0, :],
                            start=False, stop=last,
                        )
                        if SB == 0 and pi == 3:
                            # these seeds hide in the wait for group 0's
                            # k1 halves
                            seed_bias(2)
                            seed_bias(3)
                        if SB == 0 and pi == 7:
                            # and these in the wait for group 1's data
                            for gr in (4, 5, 6):
                                seed_bias(gr)

                if SB < n_sb - 1:
                    # per-group mask chains (each frees its PSUM bank);
                    # member-sum add-tree on the idle GpSimd engine
                    eqv = eqs[SB][:]
                    vote_mask(2 * SB)
                    vote_mask(2 * SB + 1)
                    t4 = tsum_pool.tile([P, 2, 4, 4, C], F16, name="t4")
                    nc.gpsimd.tensor_tensor(
                        out=t4,
                        in0=eqv[:, :, :, 0:4, :], in1=eqv[:, :, :, 4:8, :],
                        op=mybir.AluOpType.add,
                    )
                    t2 = tsum_pool.tile([P, 2, 4, 2, C], F16, name="t2")
                    nc.gpsimd.tensor_tensor(
                        out=t2,
                        in0=t4[:, :, :, 0:2, :], in1=t4[:, :, :, 2:4, :],
                        op=mybir.AluOpType.add,
                    )
                    nc.gpsimd.tensor_tensor(
                        out=stg[:, SB * 8 * C : (SB + 1) * 8 * C].rearrange(
                            "p (g t c) -> p g t c", g=2, c=C
                        ),
                        in0=t2[:, :, :, 0, :], in1=t2[:, :, :, 1, :],
                        op=mybir.AluOpType.add,
                    )
                    if SB % 2 == 1:
                        # store per SB-pair, alternating queues so no store
                        # waits behind another's issue on one engine
                        eng = nc.scalar if SB % 4 == 1 else nc.sync
                        eng.dma_start(
                            y[:, (SB - 1) * 8 * C : (SB + 1) * 8 * C],
                            stg[:, (SB - 1) * 8 * C : (SB + 1) * 8 * C],
                        )
                else:
                    nc.scalar.dma_start(
                        y[:, (SB - 1) * 8 * C : SB * 8 * C],
                        stg[:, (SB - 1) * 8 * C : SB * 8 * C],
                    )
                    # final super-batch: all-DVE per-group chains + split
                    # stores; the very last group masks at 2-tile grain so
                    # the final chain overlaps the last matmuls
                    for g in range(2):
                        if g == 0:
                            eq = vote_mask(2 * SB)
                        else:
                            gr = 2 * SB + 1
                            lg = lgs[gr]
                            eq = eqs[SB][:, 1]
                            for hf in range(2):
                                lgv = lg[
                                    :, hf * 2 * MC : (hf + 1) * 2 * MC
                                ].rearrange(
                                    "p (t m c) -> p t m c", m=M, c=C
                                )
                                mxf = mx_pool.tile([P, 2, M], F32, name="mxf")
                                nc.vector.reduce_max(
                                    mxf, lgv, axis=mybir.AxisListType.X
                                )
                                nc.vector.tensor_tensor(
                                    out=eq[:, hf * 2 : hf * 2 + 2],
                                    in0=lgv,
                                    in1=mxf[:, :, :, None].broadcast_to(
                                        [P, 2, M, C]
                                    ),
                                    op=mybir.AluOpType.is_ge,
                                )
                        t4 = tsum_pool.tile([P, 4, 4, C], F16, name="t4f")
                        nc.vector.tensor_tensor(
                            out=t4,
                            in0=eq[:, :, 0:4, :], in1=eq[:, :, 4:8, :],
                            op=mybir.AluOpType.add,
                        )
                        t2 = tsum_pool.tile([P, 4, 2, C], F16, name="t2f")
                        nc.vector.tensor_tensor(
                            out=t2,
                            in0=t4[:, :, 0:2, :], in1=t4[:, :, 2:4, :],
                            op=mybir.AluOpType.add,
                        )
                        o = SB * 8 * C + g * 4 * C
                        nc.vector.tensor_tensor(
                            out=stg[:, o : o + 4 * C].rearrange(
                                "p (t c) -> p t c", c=C
                            ),
                            in0=t2[:, :, 0, :], in1=t2[:, :, 1, :],
                            op=mybir.AluOpType.add,
                        )
                        # g0's store on scalar, g1's (the last) on sync -
                        # the two final receipts overlap
                        eng = nc.scalar if g == 0 else nc.sync
                        eng.dma_start(
                            y[:, o : o + 4 * C], stg[:, o : o + 4 * C]
                        )
    nc.compile()
    return nc


_NC_CACHE: dict[int, bass.Bass] = {}


def _get_nc(b_shard: int) -> bass.Bass:
    if b_shard not in _NC_CACHE:
        _NC_CACHE[b_shard] = build_nc(b_shard)
    return _NC_CACHE[b_shard]


def make_in_maps(x: np.ndarray, W: np.ndarray, b: np.ndarray):
    """Host-side prep: exact fp16 pair decomposition + per-core packing."""
    xf = np.asarray(x, dtype=np.float32)
    xh = xf.astype(np.float16)
    xl = (xf - xh.astype(np.float32)).astype(np.float16)
    # m-major columns: col index = 10*m + c; wh|wl concatenated per row
    wf = (
        np.asarray(W, dtype=np.float32).transpose(1, 0, 2).reshape(D, MC)
    )
    whf = wf.astype(np.float16)
    wlf = (wf - whf.astype(np.float32)).astype(np.float16)
    # packed [p, k, h, c]: row p holds (k0h0, k0h1, k1h0, k1h1) blocks
    whlf = np.empty((P, 4 * MC), dtype=np.float16)
    for k in range(2):
        for h, half in enumerate((whf, wlf)):
            whlf[:, (k * 2 + h) * MC : (k * 2 + h + 1) * MC] = half[
                k * P : (k + 1) * P
            ]
    bv = np.asarray(b, dtype=np.float32).reshape(MC)  # bv[10m+c] = b[m,c]
    bh = bv.astype(np.float16)
    bl = (bv - bh.astype(np.float32)).astype(np.float16)
    bc2 = np.ascontiguousarray(
        np.stack([np.tile(bh, 4), np.tile(bl, 4)], axis=0)
    ).astype(np.float16)

    xins = np.empty((N_CORES, P, 4 * B_SHARD), dtype=np.float16)
    halves = (xh, xl)
    for i in range(N_CORES):
        r0 = i * B_SHARD
        base = 0
        for L in CHUNKS:
            for k in range(2):
                for h in range(2):
                    c0 = 4 * base + (k * 2 + h) * L
                    xins[i, :, c0 : c0 + L] = halves[h][
                        r0 + base : r0 + base + L, k * P : (k + 1) * P
                    ].T
            base += L
    return [
        {"xin": xins[i], "whl": whlf, "bc2": bc2} for i in range(N_CORES)
    ]


def _postprocess(y_raw: np.ndarray) -> np.ndarray:
    # [p, (sb g t) * 10] fp16 -> [tile*128, 10] f32 (small ints: exact)
    n_tiles = y_raw.shape[1] // C
    return (
        y_raw.reshape(P, n_tiles, C)
        .transpose(1, 0, 2)
        .reshape(n_tiles * P, C)
        .astype(np.float32)
    )


def kernel(x: np.ndarray, W: np.ndarray, b: np.ndarray, **_) -> np.ndarray:
    from concourse.bass_utils import run_bass_kernel_spmd

    assert x.shape == (B_FULL, D), x.shape
    in_maps = make_in_maps(x, W, b)
    nc = _get_nc(B_SHARD)
    res = run_bass_kernel_spmd(nc, in_maps, core_ids=list(range(N_CORES)))
    return np.concatenate(
        [_postprocess(res.results[i]["y"]) for i in range(N_CORES)], axis=0
    )


# revision 31
# speedup vs baseline: 1.0313x; 1.0152x over previous
"""Committee-vote histogram kernel for TRN2 (8 NeuronCores, data-parallel).

votes[b, c] = sum_m 1[argmax_c' (x[b] @ W[m, :, c'] + b[m, c']) == c]

Strategy per core (batch shard of 8192 rows):
  - x is decomposed host-side into an exact fp16 pair (x = xh + xl with
    residual ~2^-22|x|); likewise W. Logits are computed as
    xh@Wh + xh@Wl + xl@Wh (+bias), whose decomposition error (~2e-7) is at
    fp32 rounding level - validated exact-match against the fp32 reference.
  - The host packs each core's x halves into ONE array [128, 4*8192] fp16
    whose rows mirror the SBUF chunk tiles exactly (per chunk: [k, h, b]
    blocks), so every chunk DMA is 128 contiguous multi-KB descriptors.
    k=0 halves stream on the sync HWDGE queue, k=1 on the scalar queue, so
    the PE's k-phased pipeline starts after the first 0.25 MB lands.
  - The first transfer on each queue ramps slowly, so chunk0 (which
    covers all of super-batch 0) is split into (k, h) quarters - k0
    halved again, consts riding between the halves - so SB0's first
    tiles gate on ~340KB of joint queue drain. While it is in flight
    the PE runs warm-up matmuls (p-state clock ramp); bias seeds (K=2
    ones @ (bh|bl) matmuls, exact f32 sum) hide inside SB0's known
    data-wait stalls, and banks 7-15 seed in the main loop on reuse.
    Middle chunks are 2048 rows so each chunk-boundary DMA-completion
    lag (~2.5us) hides behind >=3us of per-half matmul work.
  - Logits accumulate per 4-tile GROUP into single-bank PSUM tiles (7 in
    flight) so PSUM turnover is fine-grained and the PE never stalls on
    bank reuse. The steady state is LDWEIGHTS-bound (all of x flows
    through the PE weight port at ~2 loads per tile and k-half).
  - The two xh passes are FUSED into one matmul per (tile, k): rhs is the
    concatenated (wh|wl) 160 columns and the out AP carries a stride-0
    dim that folds columns 80..159 back onto 0..79, so both products
    accumulate into the same PSUM cells (start=False => every column
    write accumulates).
  - Votes: per group, DVE does reduce_max over c then an is_ge mask into
    fp16 (the second op frees the PSUM bank); the member-sum add-tree
    (exact in fp16 for counts <= 8) runs on the otherwise-idle GpSimd
    engine per super-batch, straight into the staging tile. Each super-
    batch's votes are stored with their own small DMA so only the last
    ~10KB store sits on the tail; the final super-batch runs per-group
    DVE-only chains to shorten the tail.
  - The host de-interleaves [p, sb, g, t, c] -> [b, c] and casts to f32.
"""

import os
import sys

import numpy as np

if os.path.isdir("/opt/trn_rl_repo") and "/opt/trn_rl_repo" not in sys.path:
    sys.path.insert(0, "/opt/trn_rl_repo")

import concourse.bass as bass
import concourse.tile as tile
from concourse import bacc, mybir

F32 = mybir.dt.float32
F16 = mybir.dt.float16

B_FULL = 65536
D = 256
C = 10
M = 8
N_CORES = 8
B_SHARD = B_FULL // N_CORES  # 8192
P = 128

MC = M * C  # 80 logit columns per sample
CHUNKS = (1024, 2048, 2048, 2048, 512, 512)  # rows per input-DMA chunk
WARMUP_MMS = 6


def build_nc(b_shard: int = B_SHARD) -> bass.Bass:
    assert sum(CHUNKS) == b_shard
    n_tiles = b_shard // P  # 64
    n_sb = n_tiles // 8  # 8 super-batches of 8 tiles (2 groups x 4 tiles)
    n_gr = n_tiles // 4  # 16 four-tile groups (one PSUM bank each)

    nc = bacc.Bacc("TRN2", target_bir_lowering=False)
    # packed x halves: per chunk c with L rows at column base, cols
    # [4*base + (k*2 + h)*L + 0..L) hold half (k, h) in [d-in-k, b] layout
    xin = nc.dram_tensor("xin", [P, 4 * b_shard], F16, kind="ExternalInput")
    whl = nc.dram_tensor("whl", [P, 4 * MC], F16, kind="ExternalInput")
    bc2 = nc.dram_tensor("bc2", [2, 4 * MC], F16, kind="ExternalInput")
    # votes in SBUF staging layout [p, sb*80 + g*40 + t*10 + c]
    y = nc.dram_tensor("y", [P, n_tiles * C], F16, kind="ExternalOutput")

    with tile.TileContext(nc) as tc:
        with (
            tc.tile_pool(name="consts", bufs=1) as consts,
            tc.tile_pool(name="xt", bufs=len(CHUNKS)) as xt_pool,
            tc.tile_pool(name="warm", bufs=1, space="PSUM") as warm_pool,
            tc.tile_pool(name="lg", bufs=7, space="PSUM") as lg_pool,
            tc.tile_pool(name="mx", bufs=4) as mx_pool,
            tc.tile_pool(name="eq", bufs=3) as eq_pool,
            tc.tile_pool(name="tsum", bufs=2) as tsum_pool,
            tc.tile_pool(name="stg", bufs=1) as stg_pool,
        ):
            # --- DMA issue. The first transfer on each queue ramps slowly
            # (~55-100 GB/s), so chunk0 (which covers all of SB0) is split
            # into (k, h) quarters with xh on sync and xl on scalar - both
            # queues deliver SB0's first data in parallel. bc2 (tiny) heads
            # the sync queue, whl the scalar queue.
            bc2_sb = consts.tile([2, 4 * MC], F16)
            whl_sb = consts.tile([P, 2, 2, MC], F16)
            xts = [
                xt_pool.tile([P, 2, 2, L], F16, name="xt") for L in CHUNKS
            ]
            L0 = CHUNKS[0]
            H0 = L0 // 2
            # chunk0 k0 split in half again: SB0's first four tiles gate on
            # ~340KB of joint queue drain instead of ~600KB. The consts ride
            # between the halves.
            nc.sync.dma_start(xts[0][:, 0, 0, :H0], xin[:, 0:H0])
            nc.scalar.dma_start(
                xts[0][:, 0, 1, :H0], xin[:, 2 * H0 : 3 * H0]
            )
            nc.sync.dma_start(bc2_sb, bc2[:])
            nc.scalar.dma_start(
                whl_sb, whl.rearrange("p (k h c) -> p k h c", k=2, h=2)
            )
            nc.sync.dma_start(xts[0][:, 0, 0, H0:L0], xin[:, H0 : 2 * H0])
            nc.scalar.dma_start(
                xts[0][:, 0, 1, H0:L0], xin[:, 3 * H0 : 4 * H0]
            )
            nc.sync.dma_start(
                xts[0][:, 1, 0], xin[:, 2 * L0 : 3 * L0]
            )
            nc.scalar.dma_start(
                xts[0][:, 1, 1], xin[:, 3 * L0 : 4 * L0]
            )
            base = L0
            for ci, L in list(enumerate(CHUNKS))[1:]:
                nc.sync.dma_start(
                    xts[ci][:, 0],
                    xin[:, 4 * base : 4 * base + 2 * L].rearrange(
                        "p (h l) -> p h l", h=2
                    ),
                )
                base += L
            base = L0
            for ci, L in list(enumerate(CHUNKS))[1:]:
                nc.scalar.dma_start(
                    xts[ci][:, 1],
                    xin[:, 4 * base + 2 * L : 4 * base + 4 * L].rearrange(
                        "p (h l) -> p h l", h=2
                    ),
                )
                base += L

            lgs = [
                lg_pool.tile([P, 512], F32, name="lg") for _ in range(n_gr)
            ]
            ones_g = consts.tile([P, 512], F16)
            nc.gpsimd.memset(ones_g, 1.0)

            def seed_bias(gr):
                # seed the bank with the bias: every row of ones.T @ (bh4|
                # bl4) is bh4+bl4, summed exactly in f32 PSUM
                nc.tensor.matmul(
                    lgs[gr][:, : 4 * MC], lhsT=ones_g[:2, :P], rhs=bc2_sb,
                    start=True, stop=False,
                )

            # --- PE warm-up while the first chunk is in flight; bias
            # seeds all run in the main loop (2 per super-batch) so the
            # cold-clock prologue stays short
            warm = warm_pool.tile([P, 512], F32)
            for _ in range(WARMUP_MMS):
                nc.tensor.matmul(
                    warm, lhsT=ones_g[:, :P], rhs=ones_g, start=True, stop=True
                )

            stg = stg_pool.tile([P, n_tiles * C], F16)

            # global tile T -> (chunk index, within-chunk column)
            tile_map = []
            for ci, L in enumerate(CHUNKS):
                for t in range(L // P):
                    tile_map.append((ci, t * P))

            def vote_mask(gr):
                """reduce_max + is_ge for one group; frees its PSUM bank.
                Returns the eq mask slice [P, 4, M, C] (fp16)."""
                lg = lgs[gr]
                lgv = lg[:, : 4 * MC].rearrange("p (t m c) -> p t m c", m=M, c=C)
                mx = mx_pool.tile([P, 4, M], F32, name="mx")
                nc.vector.reduce_max(mx, lgv, axis=mybir.AxisListType.X)
                eq = eqs[gr // 2][:, gr % 2]
                nc.vector.tensor_tensor(
                    out=eq,
                    in0=lgv,
                    in1=mx[:, :, :, None].broadcast_to([P, 4, M, C]),
                    op=mybir.AluOpType.is_ge,
                )
                return eq

            # --- main pipeline: super-batches of 8 tiles (2 groups) ---
            eqs = []
            for SB in range(n_sb):
                lgA, lgB = lgs[2 * SB], lgs[2 * SB + 1]
                eqs.append(eq_pool.tile([P, 2, 4, M, C], F16, name="eq"))
                if SB == 0:
                    seed_bias(0)
                    seed_bias(1)
                else:
                    for gr in (2 * SB, 2 * SB + 1):
                        if gr >= 7:
                            seed_bias(gr)
                # k-phased so phase 0 only needs the k=0 x halves
                for k in range(2):
                    for j in range(8):
                        lg = lgA if j < 4 else lgB
                        o = (j % 4) * MC
                        ci, col = tile_map[SB * 8 + j]
                        xt = xts[ci]
                        xh_c = xt[:, k, 0, col : col + P]
                        xl_c = xt[:, k, 1, col : col + P]
                        out = lg[:, o : o + MC]
                        last = k == 1 and (j % 4) == 3
                        # xh@wh + xh@wl in ONE matmul: the out AP's
                        # stride-0 h dim folds columns 80..159 onto
                        # 0..79, accumulating both products (start=False
                        # means every column-write accumulates)
                        nc.tensor.matmul(
                            out[:, None, :].broadcast_to([P, 2, MC]),
                            lhsT=xh_c, rhs=whl_sb[:, k],
                            start=False, stop=False,
                        )
                        nc.tensor.matmul(
                            out, lhsT=xl_c, rhs=whl_sb[:, k, 0, :],
                            start=False, stop=last,
                        )
                        if SB == 0 and k == 0 and j == 3:
                            # these seeds hide in the wait for chunk0's
                            # second k0 half
                            seed_bias(2)
                            seed_bias(3)
                    if SB == 0 and k == 0:
                        # and these in the wait for chunk0's k1 halves
                        for gr in (4, 5, 6):
                            seed_bias(gr)

                if SB < n_sb - 1:
                    # per-group mask chains (each frees its PSUM bank);
                    # member-sum add-tree on the idle GpSimd engine
                    eqv = eqs[SB][:]
                    vote_mask(2 * SB)
                    vote_mask(2 * SB + 1)
                    t4 = tsum_pool.tile([P, 2, 4, 4, C], F16, name="t4")
                    nc.gpsimd.tensor_tensor(
                        out=t4,
                        in0=eqv[:, :, :, 0:4, :], in1=eqv[:, :, :, 4:8, :],
                        op=mybir.AluOpType.add,
                    )
                    t2 = tsum_pool.tile([P, 2, 4, 2, C], F16, name="t2")
                    nc.gpsimd.tensor_tensor(
                        out=t2,
                        in0=t4[:, :, :, 0:2, :], in1=t4[:, :, :, 2:4, :],
                        op=mybir.AluOpType.add,
                    )
                    nc.gpsimd.tensor_tensor(
                        out=stg[:, SB * 8 * C : (SB + 1) * 8 * C].rearrange(
                            "p (g t c) -> p g t c", g=2, c=C
                        ),
                        in0=t2[:, :, :, 0, :], in1=t2[:, :, :, 1, :],
                        op=mybir.AluOpType.add,
                    )
                    if SB % 2 == 1:
                        # store per SB-pair, alternating queues so no store
                        # waits behind another's issue on one engine
                        eng = nc.scalar if SB % 4 == 1 else nc.sync
                        eng.dma_start(
                            y[:, (SB - 1) * 8 * C : (SB + 1) * 8 * C],
                            stg[:, (SB - 1) * 8 * C : (SB + 1) * 8 * C],
                        )
                else:
                    nc.scalar.dma_start(
                        y[:, (SB - 1) * 8 * C : SB * 8 * C],
                        stg[:, (SB - 1) * 8 * C : SB * 8 * C],
                    )
                    # final super-batch: all-DVE per-group chains + split
                    # stores; the very last group masks at 2-tile grain so
                    # the final chain overlaps the last matmuls
                    for g in range(2):
                        if g == 0:
                            eq = vote_mask(2 * SB)
                        else:
                            gr = 2 * SB + 1
                            lg = lgs[gr]
                            eq = eqs[SB][:, 1]
                            for hf in range(2):
                                lgv = lg[
                                    :, hf * 2 * MC : (hf + 1) * 2 * MC
                                ].rearrange(
                                    "p (t m c) -> p t m c", m=M, c=C
                                )
                                mxf = mx_pool.tile([P, 2, M], F32, name="mxf")
                                nc.vector.reduce_max(
                                    mxf, lgv, axis=mybir.AxisListType.X
                                )
                                nc.vector.tensor_tensor(
                                    out=eq[:, hf * 2 : hf * 2 + 2],
                                    in0=lgv,
                                    in1=mxf[:, :, :, None].broadcast_to(
                                        [P, 2, M, C]
                                    ),
                                    op=mybir.AluOpType.is_ge,
                                )
                        t4 = tsum_pool.tile([P, 4, 4, C], F16, name="t4f")
                        nc.vector.tensor_tensor(
                            out=t4,
                            in0=eq[:, :, 0:4, :], in1=eq[:, :, 4:8, :],
                            op=mybir.AluOpType.add,
                        )
                        t2 = tsum_pool.tile([P, 4, 2, C], F16, name="t2f")
                        nc.vector.tensor_tensor(
                            out=t2,
                            in0=t4[:, :, 0:2, :], in1=t4[:, :, 2:4, :],
                            op=mybir.AluOpType.add,
                        )
                        o = SB * 8 * C + g * 4 * C
                        nc.vector.tensor_tensor(
                            out=stg[:, o : o + 4 * C].rearrange(
                                "p (t c) -> p t c", c=C
                            ),
                            in0=t2[:, :, 0, :], in1=t2[:, :, 1, :],
                            op=mybir.AluOpType.add,
                        )
                        # g0's store on scalar, g1's (the last) on sync -
                        # the two final receipts overlap
                        eng = nc.scalar if g == 0 else nc.sync
                        eng.dma_start(
                            y[:, o : o + 4 * C], stg[:, o : o + 4 * C]
                        )
    nc.compile()
    return nc


_NC_CACHE: dict[int, bass.Bass] = {}


def _get_nc(b_shard: int) -> bass.Bass:
    if b_shard not in _NC_CACHE:
        _NC_CACHE[b_shard] = build_nc(b_shard)
    return _NC_CACHE[b_shard]


def make_in_maps(x: np.ndarray, W: np.ndarray, b: np.ndarray):
    """Host-side prep: exact fp16 pair decomposition + per-core packing."""
    xf = np.asarray(x, dtype=np.float32)
    xh = xf.astype(np.float16)
    xl = (xf - xh.astype(np.float32)).astype(np.float16)
    # m-major columns: col index = 10*m + c; wh|wl concatenated per row
    wf = (
        np.asarray(W, dtype=np.float32).transpose(1, 0, 2).reshape(D, MC)
    )
    whf = wf.astype(np.float16)
    wlf = (wf - whf.astype(np.float32)).astype(np.float16)
    # packed [p, k, h, c]: row p holds (k0h0, k0h1, k1h0, k1h1) blocks
    whlf = np.empty((P, 4 * MC), dtype=np.float16)
    for k in range(2):
        for h, half in enumerate((whf, wlf)):
            whlf[:, (k * 2 + h) * MC : (k * 2 + h + 1) * MC] = half[
                k * P : (k + 1) * P
            ]
    bv = np.asarray(b, dtype=np.float32).reshape(MC)  # bv[10m+c] = b[m,c]
    bh = bv.astype(np.float16)
    bl = (bv - bh.astype(np.float32)).astype(np.float16)
    bc2 = np.ascontiguousarray(
        np.stack([np.tile(bh, 4), np.tile(bl, 4)], axis=0)
    ).astype(np.float16)

    xins = np.empty((N_CORES, P, 4 * B_SHARD), dtype=np.float16)
    halves = (xh, xl)
    for i in range(N_CORES):
        r0 = i * B_SHARD
        base = 0
        for L in CHUNKS:
            for k in range(2):
                for h in range(2):
                    c0 = 4 * base + (k * 2 + h) * L
                    xins[i, :, c0 : c0 + L] = halves[h][
                        r0 + base : r0 + base + L, k * P : (k + 1) * P
                    ].T
            base += L
    return [
        {"xin": xins[i], "whl": whlf, "bc2": bc2} for i in range(N_CORES)
    ]


def _postprocess(y_raw: np.ndarray) -> np.ndarray:
    # [p, (sb g t) * 10] fp16 -> [tile*128, 10] f32 (small ints: exact)
    n_tiles = y_raw.shape[1] // C
    return (
        y_raw.reshape(P, n_tiles, C)
        .transpose(1, 0, 2)
        .reshape(n_tiles * P, C)
        .astype(np.float32)
    )


def kernel(x: np.ndarray, W: np.ndarray, b: np.ndarray, **_) -> np.ndarray:
    from concourse.bass_utils import run_bass_kernel_spmd

    assert x.shape == (B_FULL, D), x.shape
    in_maps = make_in_maps(x, W, b)
    nc = _get_nc(B_SHARD)
    res = run_bass_kernel_spmd(nc, in_maps, core_ids=list(range(N_CORES)))
    return np.concatenate(
        [_postprocess(res.results[i]["y"]) for i in range(N_CORES)], axis=0
    )
